# revision 10
# baseline (speedup 1.0000x reference)
"""Trainium2 Bass kernel: single-head attention (projections + masked softmax),
data-parallel over batch across 8 NeuronCores.

Per-core dataflow (one batch element per core), v2 pipeline:
  q/k/v loaded in GROUPS of 4 seq-tiles: one SWDGE cast-DMA (f32->bf16)
  [128, 4, 1024] + one 1MB contiguous xbar DMA transpose -> [128, 4, 8, 128]
  (dword-on-partitions blocks for 512 seq positions at once).
  Projections: 8 accumulating matmuls of N=512 per group (moving operand is
  the strided (i, c) chunk view), PSUM -> bf16 qsT/ksT [dk, L] via DVE.
  vs tiles masked into vs_aug [128, lt, 129] bf16 (ones-column = mask).
  Scores S^T [lk-tile, 512-lq-block] = ksT_j.T @ qsT_blk (bf16, N=512),
  two lq-blocks per PSUM pair-tile [128, 2, 512]; one wide exp ACT (N=1024)
  with fused 1/temperature scale -> es bf16.
  AV: out_aug[lq-chunk, 129] += es_chunk.T @ vs_aug_j, accumulated over all
  16 j; 8 concurrent accumulation sites packed 3-per-PSUM-bank (single
  start/stop per bank; per-element has_written handles first-write).
  Stage A: Q proj; Stage B: K/V proj fused with blocks 0-1 scores/AV
  (DMA-hidden); Stage C: blocks 2-3 scores/AV; normalize + store per block.
"""
import numpy as np

B, LQ, LK, DW, DK, DV = 8, 2048, 2048, 1024, 128, 128
TEMPERATURE = 11.313708498984761
N_CORES = 8
P = 128


def build(lq=LQ, lk=LK, dw=DW, dk=DK, dv=DV, repeat=1):
    import contextlib
    import concourse.tile as tile
    import concourse.mybir as mybir
    from concourse import bacc

    nc = bacc.Bacc("TRN2", target_bir_lowering=False, debug=False,
                   num_devices=N_CORES)
    dt = mybir.dt
    f32, bf16, i32 = dt.float32, dt.bfloat16, dt.int32
    NC = dw // P          # 8 dword chunks
    G = 4                 # seq tiles per load/transpose group
    LKt = lk // P         # 16
    NGQ = lq // (G * P)   # 4
    NGK = lk // (G * P)   # 4
    inv_t = 1.0 / TEMPERATURE

    q = nc.declare_dram_parameter("q", [lq, dw], f32, isOutput=False)
    k = nc.declare_dram_parameter("k", [lk, dw], f32, isOutput=False)
    v = nc.declare_dram_parameter("v", [lk, dw], f32, isOutput=False)
    ml = nc.declare_dram_parameter("ml", [P, 1], i32, isOutput=False)
    wq = nc.declare_dram_parameter("wq", [dw, dk], f32, isOutput=False)
    wk = nc.declare_dram_parameter("wk", [dw, dk], f32, isOutput=False)
    wv = nc.declare_dram_parameter("wv", [dw, dv], f32, isOutput=False)
    out = nc.declare_dram_parameter("out", [lq, dv], f32, isOutput=True)

    with tile.TileContext(nc) as tc:
        rep_ctx = (tc.For_i(0, repeat, 1, hint_engines=(mybir.EngineType.PE,))
                   if repeat > 1 else contextlib.nullcontext())
        with rep_ctx, \
             tc.tile_pool(name="sb", bufs=1) as sb, \
             tc.tile_pool(name="ps", bufs=1, space="PSUM") as ps:
            # sequence mask: mask[p, t] = (t*128 + p) < memory_length
            iota = sb.tile([P, LKt], i32, tag="iota")
            nc.gpsimd.iota(iota[:], pattern=[[P, LKt]], base=0,
                           channel_multiplier=1)
            mlt = sb.tile([P, 1], i32, tag="mlt")
            nc.gpsimd.dma_start(mlt[:], ml[:])
            mask = sb.tile([P, LKt], f32, tag="mask")
            nc.vector.tensor_tensor(mask[:], iota[:],
                                    mlt[:].to_broadcast([P, LKt]),
                                    mybir.AluOpType.is_lt)

            wts = {}
            for nm, src in (("wq", wq), ("wk", wk), ("wv", wv)):
                w = sb.tile([P, NC, dk], bf16, tag=nm, name=nm + "_sb")
                nc.gpsimd.dma_start(w[:], src.rearrange("(c p) d -> p c d", p=P))
                wts[nm] = w

            qsT = sb.tile([P, lq], bf16, tag="qsT")
            ksT = sb.tile([P, lk], bf16, tag="ksT")
            vsaug = sb.tile([P, LKt, dv + 1], bf16, tag="vsaug")

            H = 2 * G  # seq tiles per load DMA (half a tensor, 4MB f32 read)

            def load_half(src, h, nm):
                """One big cast-DMA: 1024 seq rows (2 transpose groups) to
                bf16 [128, 8, 1024]."""
                ld = sb.tile([P, H, dw], bf16, tag="ld", bufs=4,
                             name=f"ld_{nm}_{h}")
                nc.gpsimd.dma_start(
                    ld[:],
                    src.rearrange("(h i p) w -> h p i w", i=H, p=P)[h])
                return ld

            def transpose_group(ld, gg, nm, g):
                """xbar-transpose one 4-tile group out of a loaded half into
                [128, 4, 8, 128] (partition = dword-within-chunk; blocks are
                (seq-subtile i, dword chunk c); last dim seq-within-subtile)."""
                tb = sb.tile([P, G, NC, P], bf16, tag="tb", bufs=8,
                             name=f"tb_{nm}_{g}")
                nc.sync.dma_start_transpose(tb[:], ld[:, gg * G:(gg + 1) * G, :])
                return tb

            def load_group(src, g, nm, cache={}):
                key = (nm, g // 2)
                if key not in cache:
                    cache[key] = load_half(src, g // 2, nm)
                return transpose_group(cache[key], g % 2, nm, g)

            def proj_group(tb, w, g, nm):
                """8 accumulating matmuls of N=512 -> PSUM bank [128, 512]."""
                pp = ps.tile([P, G * P], f32, tag="pproj", bufs=1,
                             name=f"pp_{nm}_{g}")
                for c in range(NC):
                    nc.tensor.matmul(pp[:], w[:, c, :], tb[:, :, c, :],
                                     start=(c == 0), stop=(c == NC - 1))
                return pp

            # ---- Stage A: Q projections -> qsT (bf16) ----
            for g in range(NGQ):
                tb = load_group(q, g, "q")
                pp = proj_group(tb, wts["wq"], g, "q")
                nc.vector.tensor_copy(qsT[:, g * G * P:(g + 1) * G * P], pp[:])

            # avp accumulation sites: site s = blk*4 + c4 (blk pair), packed
            # 3 per PSUM bank (slot stride padded to 132 floats so sites never
            # share an 8-byte PSUM cacheline / has_written line). Single
            # start/stop per bank; per-element pending-zero handles the first
            # write of each site.
            AVW = 132
            def make_avp(phase):
                t0 = ps.tile([P, 3, AVW], f32, tag="avp0", bufs=1,
                             name=f"avp0_{phase}")
                t1 = ps.tile([P, 3, AVW], f32, tag="avp1", bufs=1,
                             name=f"avp1_{phase}")
                t2 = ps.tile([P, 2, AVW], f32, tag="avp2", bufs=1,
                             name=f"avp2_{phase}")
                tiles = [t0, t1, t2]

                def site(s):
                    return tiles[s // 3], s % 3
                return tiles, site

            def emit_av(es_t, j, site, first, last):
                """8 AV matmuls for one j: out_aug[site] += es_chunk.T @ vsaug_j.
                start only on the first matmul ever into each bank; stop only
                on the last."""
                for s in range(8):
                    t, slot = site(s)
                    nc.tensor.matmul(
                        t[:, slot, :dv + 1],
                        es_t[:, s // 4, (s % 4) * P:(s % 4 + 1) * P],
                        vsaug[:, j, :],
                        start=(first and slot == 0),
                        stop=(last and (slot == 2 or s == 7)),
                        skip_group_check=True)

            def emit_scores(j, blk_pair):
                """Two N=512 score matmuls + one wide exp ACT (N=1024)."""
                spst = ps.tile([P, 2, G * P], f32, tag="sps", bufs=2,
                               name=f"sps_{blk_pair}_{j}")
                for h in range(2):
                    blk = blk_pair * 2 + h
                    nc.tensor.matmul(
                        spst[:, h, :],
                        ksT[:, j * P:(j + 1) * P],
                        qsT[:, blk * G * P:(blk + 1) * G * P],
                        start=True, stop=True)
                es_t = sb.tile([P, 2, G * P], bf16, tag="es", bufs=3,
                               name=f"es_{blk_pair}_{j}")
                nc.scalar.activation(es_t[:], spst[:],
                                     mybir.ActivationFunctionType.Exp,
                                     scale=inv_t)
                return es_t

            def normalize_store(site, blk_pair):
                """out = out_aug[:, :dv] / out_aug[:, dv] per site; DMA out."""
                for h in range(2):
                    blk = blk_pair * 2 + h
                    osb = sb.tile([P, G, dv], f32, tag="osb", bufs=2,
                                  name=f"osb_{blk}")
                    for c4 in range(G):
                        t, slot = site(h * 4 + c4)
                        rec = sb.tile([P, 1], f32, tag="rec", bufs=4,
                                      name=f"rec_{blk}_{c4}")
                        nc.vector.reciprocal(rec[:], t[:, slot, dv:dv + 1])
                        nc.vector.tensor_scalar(osb[:, c4, :],
                                                t[:, slot, :dv],
                                                rec[:], None,
                                                mybir.AluOpType.mult)
                    nc.gpsimd.dma_start(
                        out.rearrange("(b c p) d -> b p c d", c=G, p=P)[blk],
                        osb[:])

            # ---- Stage B: K/V proj fused with blocks 0-1 scores/AV ----
            # Per-group PE emission order keeps the PE dense: the pending AV
            # matmuls run while the ksT PSUM->SBUF copy completes, and the
            # vsaug DVE ops run under the scores matmuls.
            avp01, site01 = make_avp("p01")
            prev = None  # (es_tile, j) pending AV
            for g in range(NGK):
                tbk = load_group(k, g, "k")
                tbv = load_group(v, g, "v")
                ppk = proj_group(tbk, wts["wk"], g, "k")
                nc.vector.tensor_copy(ksT[:, g * G * P:(g + 1) * G * P],
                                      ppk[:])
                if prev is not None:
                    emit_av(prev[0], prev[1], site01,
                            first=(prev[1] == 0), last=False)
                    prev = None
                # v projection in [seq, dv] orientation: stationary = tb chunk
                # (dw x seq), moving = weight chunk (dw x dv), out accumulates
                # [seq 128, dv] per sub-tile i into bank quarters.
                ppv = ps.tile([P, G * dv], f32, tag="pproj", bufs=1,
                              name=f"pp_v_{g}")
                for i in range(G):
                    for c in range(NC):
                        nc.tensor.matmul(ppv[:, i * dv:(i + 1) * dv],
                                         tbv[:, i, c, :], wts["wv"][:, c, :],
                                         start=(c == 0), stop=(c == NC - 1))
                for i in range(G):
                    j = G * g + i
                    nc.vector.tensor_scalar(vsaug[:, j, :dv],
                                            ppv[:, i * dv:(i + 1) * dv],
                                            mask[:, j:j + 1], None,
                                            mybir.AluOpType.mult)
                nc.vector.tensor_copy(vsaug[:, G * g:G * (g + 1), dv],
                                      mask[:, G * g:G * (g + 1)])
                for i in range(G):
                    j = G * g + i
                    es_t = emit_scores(j, 0)
                    if prev is not None:
                        emit_av(prev[0], prev[1], site01,
                                first=(prev[1] == 0), last=False)
                    prev = (es_t, j)
            emit_av(prev[0], prev[1], site01, first=False, last=True)
            prev = None

            normalize_store(site01, 0)

            # ---- Stage C: blocks 2-3 scores/AV ----
            avp23, site23 = make_avp("p23")
            for j in range(LKt):
                es_t = emit_scores(j, 1)
                if prev is not None:
                    emit_av(prev[0], prev[1], site23,
                            first=(prev[1] == 0), last=False)
                prev = (es_t, j)
            emit_av(prev[0], prev[1], site23, first=False, last=True)

            normalize_store(site23, 1)
    nc.compile()
    return nc


_built = None


def _get_built():
    global _built
    if _built is None:
        _built = build()
    return _built


def make_in_maps(q, k, v, memory_lengths, Wq, Wk, Wv):
    q = np.asarray(q, dtype=np.float32)
    k = np.asarray(k, dtype=np.float32)
    v = np.asarray(v, dtype=np.float32)
    ml = np.asarray(memory_lengths, dtype=np.int32)
    Wq = np.asarray(Wq, dtype=np.float32)
    Wk = np.asarray(Wk, dtype=np.float32)
    Wv = np.asarray(Wv, dtype=np.float32)
    return [
        {"q": q[b], "k": k[b], "v": v[b],
         "ml": np.full((P, 1), ml[b], dtype=np.int32),
         "wq": Wq, "wk": Wk, "wv": Wv}
        for b in range(B)
    ]


def kernel(q, k, v, memory_lengths, Wq, Wk, Wv):
    from concourse.bass_utils import run_bass_kernel_spmd
    nc = _get_built()
    in_maps = make_in_maps(q, k, v, memory_lengths, Wq, Wk, Wv)
    res = run_bass_kernel_spmd(nc, in_maps, core_ids=list(range(N_CORES)))
    return np.stack([res.results[b]["out"] for b in range(B)]).astype(np.float32)


if __name__ == "__main__":
    d = np.load("/root/problem/ref_cache.npz")
    outp = kernel(d["q"], d["k"], d["v"], d["memory_lengths"],
                  d["Wq"], d["Wk"], d["Wv"])
    exp = d["expected"]
    err = np.linalg.norm(outp - exp) / np.linalg.norm(exp)
    print("Relative error:", err)


# revision 13
# speedup vs baseline: 1.6697x; 1.6697x over previous
"""Trainium2 Bass kernel: single-head attention (projections + masked softmax),
data-parallel over batch across 8 NeuronCores.

Per-core dataflow (one batch element per core), v2 pipeline:
  q/k/v loaded in GROUPS of 4 seq-tiles: one SWDGE cast-DMA (f32->bf16)
  [128, 4, 1024] + one 1MB contiguous xbar DMA transpose -> [128, 4, 8, 128]
  (dword-on-partitions blocks for 512 seq positions at once).
  Projections: 8 accumulating matmuls of N=512 per group (moving operand is
  the strided (i, c) chunk view), PSUM -> bf16 qsT/ksT [dk, L] via DVE.
  vs tiles masked into vs_aug [128, lt, 129] bf16 (ones-column = mask).
  Scores S^T [lk-tile, 512-lq-block] = ksT_j.T @ qsT_blk (bf16, N=512),
  two lq-blocks per PSUM pair-tile [128, 2, 512]; one wide exp ACT (N=1024)
  with fused 1/temperature scale -> es bf16.
  AV: out_aug[lq-chunk, 129] += es_chunk.T @ vs_aug_j, accumulated over all
  16 j; 8 concurrent accumulation sites packed 3-per-PSUM-bank (single
  start/stop per bank; per-element has_written handles first-write).
  Stage A: Q proj; Stage B: K/V proj fused with blocks 0-1 scores/AV
  (DMA-hidden); Stage C: blocks 2-3 scores/AV; normalize + store per block.
"""
import numpy as np

B, LQ, LK, DW, DK, DV = 8, 2048, 2048, 1024, 128, 128
TEMPERATURE = 11.313708498984761
N_CORES = 8
P = 128


def build(lq=LQ, lk=LK, dw=DW, dk=DK, dv=DV, repeat=1):
    import contextlib
    import concourse.tile as tile
    import concourse.mybir as mybir
    from concourse import bacc

    nc = bacc.Bacc("TRN2", target_bir_lowering=False, debug=False,
                   num_devices=N_CORES)
    dt = mybir.dt
    f32, bf16, i32 = dt.float32, dt.bfloat16, dt.int32
    NC = dw // P          # 8 dword chunks
    G = 4                 # seq tiles per load/transpose group
    LKt = lk // P         # 16
    NGQ = lq // (G * P)   # 4
    NGK = lk // (G * P)   # 4
    inv_t = 1.0 / TEMPERATURE

    q = nc.declare_dram_parameter("q", [lq, dw], f32, isOutput=False)
    k = nc.declare_dram_parameter("k", [lk, dw], f32, isOutput=False)
    v = nc.declare_dram_parameter("v", [lk, dw], f32, isOutput=False)
    ml = nc.declare_dram_parameter("ml", [P, 1], i32, isOutput=False)
    wq = nc.declare_dram_parameter("wq", [dw, dk], f32, isOutput=False)
    wk = nc.declare_dram_parameter("wk", [dw, dk], f32, isOutput=False)
    wv = nc.declare_dram_parameter("wv", [dw, dv], f32, isOutput=False)
    out = nc.declare_dram_parameter("out", [lq, dv], f32, isOutput=True)

    with tile.TileContext(nc) as tc:
        rep_ctx = (tc.For_i(0, repeat, 1, hint_engines=(mybir.EngineType.PE,))
                   if repeat > 1 else contextlib.nullcontext())
        with rep_ctx, \
             tc.tile_pool(name="sb", bufs=1) as sb, \
             tc.tile_pool(name="ps", bufs=1, space="PSUM") as ps:
            # sequence mask: mask[p, t] = (t*128 + p) < memory_length
            iota = sb.tile([P, LKt], i32, tag="iota")
            nc.gpsimd.iota(iota[:], pattern=[[P, LKt]], base=0,
                           channel_multiplier=1)
            mlt = sb.tile([P, 1], i32, tag="mlt")
            nc.gpsimd.dma_start(mlt[:], ml[:])
            mask = sb.tile([P, LKt], f32, tag="mask")
            nc.vector.tensor_tensor(mask[:], iota[:],
                                    mlt[:].to_broadcast([P, LKt]),
                                    mybir.AluOpType.is_lt)

            wts = {}
            for nm, src in (("wq", wq), ("wk", wk), ("wv", wv)):
                w = sb.tile([P, NC, dk], bf16, tag=nm, name=nm + "_sb")
                nc.gpsimd.dma_start(w[:], src.rearrange("(c p) d -> p c d", p=P))
                wts[nm] = w

            qsT = sb.tile([P, lq], bf16, tag="qsT")
            ksT = sb.tile([P, lk], bf16, tag="ksT")
            vsaug = sb.tile([P, LKt, dv + 1], bf16, tag="vsaug")

            H = 2 * G  # seq tiles per load DMA / transpose (half a tensor)

            def load_half(src, h, nm):
                """One 4MB cast-DMA (1024 seq rows -> bf16 [128, 8, 1024]) and
                one 2MB xbar transpose -> [128, 8, 8, 128] (partition = dword-
                within-chunk; blocks are (seq-subtile i, dword chunk c))."""
                ld = sb.tile([P, H, dw], bf16, tag="ld", bufs=3,
                             name=f"ld_{nm}_{h}")
                nc.gpsimd.dma_start(
                    ld[:],
                    src.rearrange("(h i p) w -> h p i w", i=H, p=P)[h])
                tb = sb.tile([P, H, NC, P], bf16, tag="tb", bufs=4,
                             name=f"tb_{nm}_{h}")
                nc.sync.dma_start_transpose(tb[:], ld[:])
                return tb

            def load_group(src, g, nm, cache={}):
                """Return a [128, 4, 8, 128]-slice view of the half-tensor
                transposed tile covering seq group g."""
                key = (nm, g // 2)
                if key not in cache:
                    cache[key] = load_half(src, g // 2, nm)
                i0 = (g % 2) * G
                return cache[key][:, i0:i0 + G]

            def proj_group(tb, w, g, nm):
                """8 accumulating matmuls of N=512 -> PSUM bank [128, 512]."""
                pp = ps.tile([P, G * P], f32, tag="pproj", bufs=1,
                             name=f"pp_{nm}_{g}")
                for c in range(NC):
                    nc.tensor.matmul(pp[:], w[:, c, :], tb[:, :, c, :],
                                     start=(c == 0), stop=(c == NC - 1))
                return pp

            # ---- Stage A: Q projections -> qsT (bf16) ----
            for g in range(NGQ):
                tb = load_group(q, g, "q")
                pp = proj_group(tb, wts["wq"], g, "q")
                nc.vector.tensor_copy(qsT[:, g * G * P:(g + 1) * G * P], pp[:])

            # avp accumulation sites: site s = blk*4 + c4 (blk pair), packed
            # 3 per PSUM bank (slot stride padded to 132 floats so sites never
            # share an 8-byte PSUM cacheline / has_written line). Single
            # start/stop per bank; per-element pending-zero handles the first
            # write of each site.
            AVW = 132
            def make_avp(phase):
                t0 = ps.tile([P, 3, AVW], f32, tag="avp0", bufs=1,
                             name=f"avp0_{phase}")
                t1 = ps.tile([P, 3, AVW], f32, tag="avp1", bufs=1,
                             name=f"avp1_{phase}")
                t2 = ps.tile([P, 2, AVW], f32, tag="avp2", bufs=1,
                             name=f"avp2_{phase}")
                tiles = [t0, t1, t2]

                def site(s):
                    return tiles[s // 3], s % 3
                return tiles, site

            def emit_av(es_t, j, site, first, last):
                """8 AV matmuls for one j: out_aug[site] += es_chunk.T @ vsaug_j.
                start only on the first matmul ever into each bank; stop only
                on the last."""
                for s in range(8):
                    t, slot = site(s)
                    nc.tensor.matmul(
                        t[:, slot, :dv + 1],
                        es_t[:, s // 4, (s % 4) * P:(s % 4 + 1) * P],
                        vsaug[:, j, :],
                        start=(first and slot == 0),
                        stop=(last and (slot == 2 or s == 7)),
                        skip_group_check=True)

            def emit_scores(j, blk_pair):
                """Two N=512 score matmuls + one wide exp ACT (N=1024)."""
                spst = ps.tile([P, 2, G * P], f32, tag="sps", bufs=2,
                               name=f"sps_{blk_pair}_{j}")
                for h in range(2):
                    blk = blk_pair * 2 + h
                    nc.tensor.matmul(
                        spst[:, h, :],
                        ksT[:, j * P:(j + 1) * P],
                        qsT[:, blk * G * P:(blk + 1) * G * P],
                        start=True, stop=True)
                es_t = sb.tile([P, 2, G * P], bf16, tag="es", bufs=3,
                               name=f"es_{blk_pair}_{j}")
                nc.scalar.activation(es_t[:], spst[:],
                                     mybir.ActivationFunctionType.Exp,
                                     scale=inv_t)
                return es_t

            def normalize_store(site, blk_pair):
                """out = out_aug[:, :dv] / out_aug[:, dv] per site; DMA out."""
                for h in range(2):
                    blk = blk_pair * 2 + h
                    osb = sb.tile([P, G, dv], f32, tag="osb", bufs=2,
                                  name=f"osb_{blk}")
                    for c4 in range(G):
                        t, slot = site(h * 4 + c4)
                        rec = sb.tile([P, 1], f32, tag="rec", bufs=4,
                                      name=f"rec_{blk}_{c4}")
                        nc.vector.reciprocal(rec[:], t[:, slot, dv:dv + 1])
                        nc.vector.tensor_scalar(osb[:, c4, :],
                                                t[:, slot, :dv],
                                                rec[:], None,
                                                mybir.AluOpType.mult)
                    nc.gpsimd.dma_start(
                        out.rearrange("(b c p) d -> b p c d", c=G, p=P)[blk],
                        osb[:])

            # ---- Stage B: K/V proj fused with blocks 0-1 scores/AV ----
            # Per-group PE emission order keeps the PE dense: the pending AV
            # matmuls run while the ksT PSUM->SBUF copy completes, and the
            # vsaug DVE ops run under the scores matmuls.
            avp01, site01 = make_avp("p01")
            prev = None  # (es_tile, j) pending AV
            for g in range(NGK):
                tbk = load_group(k, g, "k")
                tbv = load_group(v, g, "v")
                ppk = proj_group(tbk, wts["wk"], g, "k")
                nc.vector.tensor_copy(ksT[:, g * G * P:(g + 1) * G * P],
                                      ppk[:])
                if prev is not None:
                    emit_av(prev[0], prev[1], site01,
                            first=(prev[1] == 0), last=False)
                    prev = None
                # v projection in [seq, dv] orientation: stationary = tb chunk
                # (dw x seq), moving = weight chunk (dw x dv), out accumulates
                # [seq 128, dv] per sub-tile i into bank quarters.
                ppv = ps.tile([P, G * dv], f32, tag="pproj", bufs=1,
                              name=f"pp_v_{g}")
                for i in range(G):
                    for c in range(NC):
                        nc.tensor.matmul(ppv[:, i * dv:(i + 1) * dv],
                                         tbv[:, i, c, :], wts["wv"][:, c, :],
                                         start=(c == 0), stop=(c == NC - 1))
                for i in range(G):
                    j = G * g + i
                    nc.vector.tensor_scalar(vsaug[:, j, :dv],
                                            ppv[:, i * dv:(i + 1) * dv],
                                            mask[:, j:j + 1], None,
                                            mybir.AluOpType.mult)
                nc.vector.tensor_copy(vsaug[:, G * g:G * (g + 1), dv],
                                      mask[:, G * g:G * (g + 1)])
                for i in range(G):
                    j = G * g + i
                    es_t = emit_scores(j, 0)
                    if prev is not None:
                        emit_av(prev[0], prev[1], site01,
                                first=(prev[1] == 0), last=False)
                    prev = (es_t, j)
            emit_av(prev[0], prev[1], site01, first=False, last=True)
            prev = None

            normalize_store(site01, 0)

            # ---- Stage C: blocks 2-3 scores/AV ----
            avp23, site23 = make_avp("p23")
            for j in range(LKt):
                es_t = emit_scores(j, 1)
                if prev is not None:
                    emit_av(prev[0], prev[1], site23,
                            first=(prev[1] == 0), last=False)
                prev = (es_t, j)
            emit_av(prev[0], prev[1], site23, first=False, last=True)

            normalize_store(site23, 1)
    nc.compile()
    return nc


_built = None


def _get_built():
    global _built
    if _built is None:
        _built = build()
    return _built


def make_in_maps(q, k, v, memory_lengths, Wq, Wk, Wv):
    q = np.asarray(q, dtype=np.float32)
    k = np.asarray(k, dtype=np.float32)
    v = np.asarray(v, dtype=np.float32)
    ml = np.asarray(memory_lengths, dtype=np.int32)
    Wq = np.asarray(Wq, dtype=np.float32)
    Wk = np.asarray(Wk, dtype=np.float32)
    Wv = np.asarray(Wv, dtype=np.float32)
    return [
        {"q": q[b], "k": k[b], "v": v[b],
         "ml": np.full((P, 1), ml[b], dtype=np.int32),
         "wq": Wq, "wk": Wk, "wv": Wv}
        for b in range(B)
    ]


def kernel(q, k, v, memory_lengths, Wq, Wk, Wv):
    from concourse.bass_utils import run_bass_kernel_spmd
    nc = _get_built()
    in_maps = make_in_maps(q, k, v, memory_lengths, Wq, Wk, Wv)
    res = run_bass_kernel_spmd(nc, in_maps, core_ids=list(range(N_CORES)))
    return np.stack([res.results[b]["out"] for b in range(B)]).astype(np.float32)


if __name__ == "__main__":
    d = np.load("/root/problem/ref_cache.npz")
    outp = kernel(d["q"], d["k"], d["v"], d["memory_lengths"],
                  d["Wq"], d["Wk"], d["Wv"])
    exp = d["expected"]
    err = np.linalg.norm(outp - exp) / np.linalg.norm(exp)
    print("Relative error:", err)
